# revision 7
# baseline (speedup 1.0000x reference)
"""Multi-head attention (B=2, L=2048, D=1024, H=16) on 8 Trainium2 cores. v4.

Sharding: core = b * 4 + g (b data-parallel, g = head-group of 4 heads).

Schedule (vs v3): one flat PE pipeline where every PE psum output (scores,
V/Q/O projection groups, drain broadcast) rotates through a single 3-deep
PSUM ring [128, 3, 1024] (6 banks), with ps_o [65, 1024] (2 banks) as the
PV accumulator. Structure per phase p (16 slots, one lk-tile of scores
each):
  - scores of head HP[p] -> ring slot; exp of the previous slot issued
    right after (ACT; ~1.1us each, 16/phase ~= the PE phase time).
  - drain of HP[p-2] threaded in at slots 0-3: ps_o copy (DVE), recip
    (DVE), ones-broadcast (PE ring slot) + ot multiply (DVE) -- PE never
    waits on the DVE chain.
  - PV of HP[p-1] occupies slots 4..15 (ps_o freed by the slot-0 copy).
  - fillers keep PE >= ACT per phase: V-proj ph 0-1, Q-proj half1 ph 2-3,
    O-proj ch0 ph 5-7, rest in the tail.
  - startup: K-proj, Q-proj(half0) through the same ring; xk/xq lead the
    SP/Act HWDGE queues so PE starts ~7us in.
"""
import sys

sys.path.insert(0, "/opt/trn_rl_repo")

import numpy as np

import concourse.bass as bass
import concourse.bacc as bacc
import concourse.mybir as mybir
import concourse.tile as tile
from concourse import bass_utils

F32R = mybir.dt.float32r
F32 = mybir.dt.float32
BF16 = mybir.dt.bfloat16
AF = mybir.ActivationFunctionType

B, L, D = 2, 2048, 1024
H, HD = 16, 64
G = 4
GD = D // G              # 256 projection rows per group
SCALE = HD ** -0.5
NKT = D // 128           # 8 contraction tiles for projections
NLK = L // 128           # 16 k-tiles in attention
P = 128
NW = 4                   # psum half-window ring depth (4 x 512 = 2 banks each lk)


def _to_fp32r(x):
    u = np.ascontiguousarray(x, dtype=np.float32).view(np.uint32).copy()
    lsb = (u >> 12) & 1
    u += 0x7FF + lsb
    u &= 0xFFFFF000
    return u.view(np.float32)


def _build(dbg=False, repeat=1):
    nc = bacc.Bacc("TRN2", target_bir_lowering=False, debug=False, num_devices=8)

    xqT = nc.dram_tensor("xqT", [D, L], BF16, kind="ExternalInput")
    xkT = nc.dram_tensor("xkT", [D, L], BF16, kind="ExternalInput")
    xvT = nc.dram_tensor("xvT", [D, L], BF16, kind="ExternalInput")
    wqT = nc.dram_tensor("wqT", [D, GD], BF16, kind="ExternalInput")
    wkT = nc.dram_tensor("wkT", [D, GD], BF16, kind="ExternalInput")
    wvT = nc.dram_tensor("wvT", [D, GD], BF16, kind="ExternalInput")
    woT = nc.dram_tensor("woT", [GD, D], BF16, kind="ExternalInput")
    bqv = nc.dram_tensor("bqv", [P, GD // P], F32, kind="ExternalInput")
    bkv = nc.dram_tensor("bkv", [P, GD // P], F32, kind="ExternalInput")
    bvv = nc.dram_tensor("bvv", [1, GD], F32, kind="ExternalInput")
    m01f = nc.dram_tensor("m01f", [P, NLK], F32, kind="ExternalInput")
    ones64 = nc.dram_tensor("ones64", [1, HD], BF16, kind="ExternalInput")
    out = nc.dram_tensor("out", [L, D], BF16, kind="ExternalOutput")

    with tile.TileContext(nc) as tc, \
            nc.allow_low_precision(reason="bf16/fp32r matmul pipeline, fp32 PSUM"):
        with tc.tile_pool(name="wp", bufs=1) as wp, \
                tc.tile_pool(name="cn", bufs=1) as cn, \
                tc.tile_pool(name="xp", bufs=2) as xp, \
                tc.tile_pool(name="big", bufs=1) as big, \
                tc.tile_pool(name="on", bufs=1) as onp, \
                tc.tile_pool(name="sm", bufs=1) as smp, \
                tc.tile_pool(name="ob", bufs=2) as obp, \
                tc.tile_pool(name="sw", bufs=1, space="PSUM") as swp, \
                tc.tile_pool(name="fl", bufs=1, space="PSUM") as flp, \
                tc.tile_pool(name="po", bufs=1, space="PSUM") as pop:

            rep_cm = tc.For_i(0, repeat, 1) if repeat > 1 else None
            if rep_cm is not None:
                rep_cm.__enter__()

            # ---- DMA: Act queue = wk/wq/consts, xq halves, wv/wo;
            # SP queue = xk halves, xv halves, out chunks.
            wq_t = wp.tile([P, NKT, GD], BF16, tag="wq")
            wk_t = wp.tile([P, NKT, GD], BF16, tag="wk")
            wv_t = wp.tile([P, NKT, GD], BF16, tag="wv")
            wo_t = wp.tile([P, GD // P, D], BF16, tag="wo")
            bq_t = cn.tile([P, GD // P], F32, tag="bq")
            bk_t = cn.tile([P, GD // P], F32, tag="bk")
            bvb_t = cn.tile([P, G, HD], F32, tag="bvb")
            m01f_t = cn.tile([P, NLK], F32, tag="m01f")
            ones64_t = cn.tile([1, HD], BF16, tag="ones64")

            nc.scalar.dma_start(
                out=wk_t, in_=wkT.ap().rearrange("(kt p) m -> p kt m", p=P))
            nc.scalar.dma_start(
                out=wq_t, in_=wqT.ap().rearrange("(kt p) m -> p kt m", p=P))
            nc.scalar.dma_start(out=bk_t, in_=bkv.ap())
            nc.scalar.dma_start(out=bq_t, in_=bqv.ap())
            nc.scalar.dma_start(out=m01f_t, in_=m01f.ap())
            nc.scalar.dma_start(out=ones64_t, in_=ones64.ap())
            _bv = bvv.ap()
            nc.scalar.dma_start(
                out=bvb_t,
                in_=bass.AP(tensor=_bv.tensor, offset=_bv.offset,
                            ap=[[0, P], [HD, G], [1, HD]]))

            # xk/xv share a 2-buf ring (xv DMA waits K-proj of the same
            # half -- done long before v_chunks need it); xq gets its own
            # 2 buffers so Q-proj never waits on a WAR.
            def x_dma(eng, src, half, tag):
                x_t = xp.tile([P, NKT, L // 2], BF16, tag=tag)
                eng.dma_start(
                    out=x_t,
                    in_=src.ap().rearrange("(kt p) n -> p kt n", p=P)
                    [:, :, half * (L // 2):(half + 1) * (L // 2)])
                return x_t
            xk_tiles = [x_dma(nc.sync, xkT, 0, "xkv"),
                        x_dma(nc.sync, xkT, 1, "xkv")]
            xq_tiles = [x_dma(nc.scalar, xqT, 0, "xq"),
                        x_dma(nc.scalar, xqT, 1, "xq")]
            nc.scalar.dma_start(
                out=wv_t, in_=wvT.ap().rearrange("(kt p) m -> p kt m", p=P))
            nc.scalar.dma_start(
                out=wo_t, in_=woT.ap().rearrange("(kt p) m -> p kt m", p=P))

            # ---- persistent tiles ----
            qt_t = big.tile([P, 2, L], BF16, tag="qt")   # Q.T (d rows, q)
            kt_t = big.tile([P, 2, L], BF16, tag="kt")   # K.T
            ot_t = big.tile([P, 2, L], BF16, tag="ot")   # attention out .T
            va_t = big.tile([P, NLK, G, HD + 1], BF16, tag="va")
            tt_a = big.tile([P, NLK, 1024], BF16, tag="tta")
            tt_b = big.tile([P, NLK, 1024], BF16, tag="ttb")
            tt_bufs = [tt_a, tt_b]
            on_a = big.tile([HD + 1, 1024], F32, tag="ona")
            on_b = big.tile([HD + 1, 1024], F32, tag="onb")
            on_bufs = [on_a, on_b]
            den0_t = big.tile([1, 1024], F32, tag="den0s")
            rec_t = big.tile([1, 1024], F32, tag="recs")
            recr_t = big.tile([1, 1024], BF16, tag="recrs")
            ob_a = big.tile([P, D], BF16, tag="oba")
            ob_b = big.tile([P, D], BF16, tag="obb")
            ob_bufs = [ob_a, ob_b]
            dri = {"i": 0}
            # denominator column: starts at 1, the per-lkt mask multiply
            # below zeroes it (with the V values) for masked k rows
            nc.vector.memset(va_t[:, :, :, HD:HD + 1], 1.0)

            ps_o = pop.tile([HD + 1, 1024], F32, tag="po")    # 2 banks
            win_a = swp.tile([P, 1024], F32, tag="wina")
            win_b = swp.tile([P, 1024], F32, tag="winb")
            win_bufs = [win_a, win_b]
            wni = {"i": 0}

            def win_tile():
                w = win_bufs[wni["i"] % 2]
                wni["i"] += 1
                return w
            fl_a = flp.tile([P, 512], F32, tag="fla")
            fl_b = flp.tile([P, 512], F32, tag="flb")
            fl_bufs = [fl_a, fl_b]
            fli = {"i": 0}

            def fl_tile():
                t = fl_bufs[fli["i"] % 2]
                fli["i"] += 1
                return t

            # ---- startup projections (through the filler pool) ----
            def proj_qk(x_tile, w_tile, b_tile, dst, half):
                col = half * (L // 2)
                for mt in range(2):
                    for qc in range(2):
                        reg = fl_tile()
                        for kt in range(NKT):
                            nc.tensor.matmul(
                                reg[:, 0:512],
                                w_tile[:, kt, mt * P:(mt + 1) * P],
                                x_tile[:, kt, qc * 512:(qc + 1) * 512],
                                start=(kt == 0), stop=(kt == NKT - 1))
                        nc.vector.tensor_scalar_add(
                            dst[:, mt, col + qc * 512:col + (qc + 1) * 512],
                            reg[:, 0:512], b_tile[:, mt:mt + 1])

            proj_qk(xk_tiles[0], wk_t, bk_t, kt_t, 0)
            proj_qk(xk_tiles[1], wk_t, bk_t, kt_t, 1)
            xv_tiles = [x_dma(nc.sync, xvT, 0, "xkv"),
                        x_dma(nc.sync, xvT, 1, "xkv")]
            proj_qk(xq_tiles[0], wq_t, bq_t, qt_t, 0)

            def qproj_h1_group(gi):
                mt, qc = gi // 2, gi % 2
                reg = fl_tile()
                for kt in range(NKT):
                    nc.tensor.matmul(
                        reg[:, 0:512],
                        wq_t[:, kt, mt * P:(mt + 1) * P],
                        xq_tiles[1][:, kt, qc * 512:(qc + 1) * 512],
                        start=(kt == 0), stop=(kt == NKT - 1))
                nc.vector.tensor_scalar_add(
                    qt_t[:, mt, 1024 + qc * 512:1024 + (qc + 1) * 512],
                    reg[:, 0:512], bq_t[:, mt:mt + 1])

            def v_chunk(lkt):
                x_t = xv_tiles[lkt // (NLK // 2)]
                loc = lkt % (NLK // 2)
                reg = fl_tile()
                for kt in range(NKT):
                    nc.tensor.matmul(
                        reg[:, 0:GD], x_t[:, kt, loc * P:(loc + 1) * P],
                        wv_t[:, kt, :],
                        start=(kt == 0), stop=(kt == NKT - 1))
                nc.vector.tensor_tensor(
                    out=va_t[:, lkt, :, 0:HD],
                    in0=reg[:, 0:GD].rearrange("p (h d) -> p h d", h=G),
                    in1=bvb_t[:],
                    op=mybir.AluOpType.add)
                nc.vector.tensor_scalar_mul(
                    va_t[:, lkt, :, :], va_t[:, lkt, :, :],
                    m01f_t[:, lkt:lkt + 1])

            def o_proj(qa, act_copy=False):
                ob = ob_bufs[qa % 2]
                pair = [fl_a, fl_b]
                for kt in range(2):
                    for nch in range(2):
                        nc.tensor.matmul(
                            pair[nch][:],
                            ot_t[:, kt, qa * P:(qa + 1) * P],
                            wo_t[:, kt, nch * 512:(nch + 1) * 512],
                            start=(kt == 0), stop=(kt == 1),
                            skip_group_check=True)
                for nch in range(2):
                    if act_copy:
                        nc.scalar.activation(
                            ob[:, nch * 512:(nch + 1) * 512], pair[nch][:],
                            AF.Copy, scale=1.0)
                    else:
                        nc.vector.tensor_copy(
                            ob[:, nch * 512:(nch + 1) * 512], pair[nch][:])
                nc.sync.dma_start(out=out.ap()[qa * P:(qa + 1) * P, :], in_=ob[:])

            # ---- drain pieces (head whose PV finished last phase) ----
            def drain_copy(state):
                onum = on_bufs[dri["i"] % 2]
                dri["i"] += 1
                nc.vector.tensor_copy(onum[:], ps_o[:])
                state["onum"] = onum

            def drain_recip(state):
                nc.vector.tensor_copy(den0_t[:], state["onum"][HD:HD + 1, :])
                nc.vector.reciprocal_approx_fast(rec_t[:], den0_t[:])
                nc.vector.tensor_copy(recr_t[:], rec_t[:])
                state["recr"] = recr_t

            def drain_bcast(state):
                pair = [fl_a, fl_b]
                for sc in range(2):
                    nc.tensor.matmul(pair[sc][0:HD, :],
                                     ones64_t[:],
                                     state["recr"][:, sc * 512:(sc + 1) * 512],
                                     start=True, stop=True)
                h, ch = state["head"]
                mt, po = h // 2, (h % 2) * HD
                for sc in range(2):
                    nc.vector.tensor_mul(
                        ot_t[po:po + HD, mt,
                             ch * 1024 + sc * 512:ch * 1024 + (sc + 1) * 512],
                        state["onum"][0:HD, sc * 512:(sc + 1) * 512],
                        pair[sc][0:HD, :])

            # ---- main attention pipeline ----
            # heads ordered 0,2,1,3: adjacent phases keep the same
            # partition range (po) for kt/qt operands -- the PE pays a
            # penalty when operand base partitions alternate 0/64.
            HP = [(c, hh) for c in range(2) for hh in (0, 2, 1, 3)]

            def scores_lk(ch, h, lk, win):
                mt, po = h // 2, (h % 2) * HD
                for c in range(2):
                    nc.tensor.matmul(
                        win[:, c * 512:(c + 1) * 512],
                        kt_t[po:po + HD, mt, lk * P:(lk + 1) * P],
                        qt_t[po:po + HD, mt,
                             ch * 1024 + c * 512:ch * 1024 + (c + 1) * 512],
                        start=True, stop=True, skip_group_check=True)

            def pv_lk(ph, pch, lk, tt_prev):
                for sc in range(2):
                    nc.tensor.matmul(
                        ps_o[:, sc * 512:(sc + 1) * 512],
                        va_t[:, lk, ph, :],
                        tt_prev[:, lk, sc * 512:(sc + 1) * 512],
                        start=(lk == 0), stop=(lk == NLK - 1),
                        skip_group_check=True)

            # PV lk-tiles per slot (slots 2..15)
            PV_N = [0, 0, 1, 1, 1, 1, 1, 1, 1, 1, 1, 1, 2, 2, 1, 1]
            assert sum(PV_N) == NLK

            # fillers: phase -> {slot -> [callable]}
            fillers = {p: {} for p in range(9)}
            for i in range(12):
                fillers[0].setdefault(4 + i, []).append(
                    (lambda lk: lambda: v_chunk(lk))(i))
            for i in range(4):
                fillers[1].setdefault(i, []).append(
                    (lambda lk: lambda: v_chunk(lk))(12 + i))
            for gi in range(4):
                fillers[2 + gi // 2].setdefault(2 + (gi % 2) * 8, []).append(
                    (lambda g: lambda: qproj_h1_group(g))(gi))
            oqa = 0
            for p in (5, 6, 7):
                for s in (5, 10, 14):
                    if oqa < 8:
                        fillers[p].setdefault(s, []).append(
                            (lambda q: lambda: o_proj(q))(oqa))
                        oqa += 1

            prev = None          # (h, ch, tt buffer) whose PV runs this phase
            dr = None            # drain state for head 2 phases back
            for p in range(9):
                is_sc = p < 8
                if is_sc:
                    ch, h = HP[p]
                    tt_cur = tt_bufs[p % 2]
                pend = None      # (lk, win tile) awaiting exp
                pv_i = 0
                for s in range(16):
                    prev_pend, pend = pend, None
                    if is_sc:
                        win = win_tile()
                        scores_lk(ch, h, s, win)
                        pend = (s, win)
                    if prev_pend is not None:
                        nc.scalar.activation(
                            tt_cur[:, prev_pend[0], :], prev_pend[1][:],
                            AF.Exp, scale=SCALE)
                    if dr is not None:
                        if s == 0:
                            drain_copy(dr)
                        elif s == 1:
                            drain_recip(dr)
                        elif s == 3:
                            drain_bcast(dr)
                            dr = None
                    for f in fillers[p].get(s, []):
                        f()
                    if prev is not None:
                        for _ in range(PV_N[s]):
                            pv_lk(prev[0], prev[1], pv_i, prev[2])
                            pv_i += 1
                if pend is not None:
                    nc.scalar.activation(
                        tt_cur[:, pend[0], :], pend[1][:],
                        AF.Exp, scale=SCALE)
                if prev is not None:
                    assert pv_i == NLK
                    dr = {"head": (prev[0], prev[1])}
                prev = (h, ch, tt_cur) if is_sc else None
            # tail: drain of HP[7], then remaining o_proj
            drain_copy(dr)
            drain_recip(dr)
            drain_bcast(dr)
            # tail: ACT is idle after the last exp -- alternate the
            # PSUM->SBUF copies between DVE and ACT so neither serializes
            for qa in range(8, 16):
                o_proj(qa, act_copy=(qa % 2 == 1))

            if rep_cm is not None:
                rep_cm.__exit__(None, None, None)

    nc.compile()
    return nc


_NC = None


def _get_nc():
    global _NC
    if _NC is None:
        _NC = _build()
    return _NC


def _build_in_maps(q, k, v, kv_mask, Wq, bq, Wk, bk, Wv, bv, Wo, bo):
    import ml_dtypes
    bf = ml_dtypes.bfloat16
    q = np.asarray(q, np.float32)
    k = np.asarray(k, np.float32)
    v = np.asarray(v, np.float32)
    kv_mask = np.asarray(kv_mask)
    Wq, bq = np.asarray(Wq, np.float32), np.asarray(bq, np.float32)
    Wk, bk = np.asarray(Wk, np.float32), np.asarray(bk, np.float32)
    Wv, bv = np.asarray(Wv, np.float32), np.asarray(bv, np.float32)
    Wo = np.asarray(Wo, np.float32)

    xT = {b: {"q": np.ascontiguousarray(q[b].T).astype(bf),
              "k": np.ascontiguousarray(k[b].T).astype(bf),
              "v": np.ascontiguousarray(v[b].T).astype(bf)}
          for b in range(B)}
    m01 = {b: np.ascontiguousarray(
               (kv_mask[b] != 0).astype(np.float32).reshape(NLK, P).T)
           for b in range(B)}
    ones64 = np.ones((1, HD), bf)

    wslice = {}
    for g in range(G):
        rows = slice(g * GD, (g + 1) * GD)
        wslice[g] = {
            "wqT": np.ascontiguousarray(Wq[rows, :].T).astype(bf),
            "wkT": np.ascontiguousarray(Wk[rows, :].T).astype(bf),
            "wvT": np.ascontiguousarray(Wv[rows, :].T).astype(bf),
            "woT": np.ascontiguousarray(Wo[:, rows].T).astype(bf),
            "bqv": np.ascontiguousarray(bq[rows].reshape(GD // P, P).T),
            "bkv": np.ascontiguousarray(bk[rows].reshape(GD // P, P).T),
            "bvv": bv[rows].reshape(1, GD),
        }

    in_maps = []
    for core in range(8):
        b, g = core // G, core % G
        m = {"xqT": xT[b]["q"], "xkT": xT[b]["k"], "xvT": xT[b]["v"],
             "m01f": m01[b], "ones64": ones64}
        m.update(wslice[g])
        in_maps.append(m)
    return in_maps


def kernel(q, k, v, kv_mask, Wq, bq, Wk, bk, Wv, bv, Wo, bo):
    bo = np.asarray(bo, np.float32)
    in_maps = _build_in_maps(q, k, v, kv_mask, Wq, bq, Wk, bk, Wv, bv, Wo, bo)
    nc = _get_nc()
    res = bass_utils.run_bass_kernel_spmd(nc, in_maps, core_ids=list(range(8)))

    outs = [r["out"] for r in res.results]
    full = np.empty((B, L, D), np.float32)
    for b in range(B):
        acc = outs[b * G].astype(np.float32).copy()
        for g in range(1, G):
            acc += outs[b * G + g]
        full[b] = acc + bo[None, :]
    return full
